# revision 12
# baseline (speedup 1.0000x reference)
"""AttnBlock (GroupNorm + single-head self-attention + residual) on 8 TRN2 cores.

Shapes (hardcoded): x [2, 128, 16, 16, 16] fp32 -> [B=2, C=128, N=4096].

Sharding: sequence-parallel over the N=4096 query dim, 4 cores per batch
(8 cores total). Each core receives its batch's x rolled so that its
1024 query columns sit at columns 0:1024; K/V are recomputed from the
full rolled x on every core (no collectives needed).

Per-core program:
  GN stats (bn_stats/bn_aggr + group-combine matmul) -> h = GN(x) in bf16
  K = wk h + bk  [C, 4096]   Q' = (wq/sqrt(C)) h + bq' [C, 1024]
  V^T tiles [keys, C] with an appended ones column (denominator trick)
  S^T tiles = K_tile^T Q'  -> exp (no max-subtraction; inputs are bounded)
  O_raw[q, 0:128] + den[q] = sum_k expS^T[k,q] * [V^T | 1]
  O = O_raw/den (per-partition), PE-transpose to [C, q], + bv
  out = x + wp O + bp  -> DMA [128, 1024] slice
"""

import os
import sys

import numpy as np

for _p in ("/opt/trn_rl_repo", "/root/.axon_site/_ro/trn_rl_repo"):
    if os.path.isdir(_p) and _p not in sys.path:
        sys.path.insert(0, _p)

import concourse.bass as bass
import concourse.tile as tile
from concourse import bacc, mybir
from concourse.bass_utils import run_bass_kernel_spmd
from concourse.masks import make_identity

F32 = mybir.dt.float32
BF16 = mybir.dt.bfloat16
AF = mybir.ActivationFunctionType
OP = mybir.AluOpType

B, C, N = 2, 128, 4096
NQ = 1024  # query columns per core
NCORES = 8
GROUPS = 32
EPS = 1e-5


def _emit_o_group(nc, opool, oacc, ptiles, vt_sb, g):
    """O accumulation for key-group g (8 key tiles) using its exp(S^T) tiles."""
    for qs8 in range(8):
        o_ps = opool.tile([128, 129], F32, tag="o")
        for j in range(8):
            nc.tensor.matmul(
                o_ps[:],
                lhsT=ptiles[j][:, qs8 * 128 : (qs8 + 1) * 128],
                rhs=vt_sb[:, g * 8 + j, :],
                start=(j == 0),
                stop=(j == 7),
            )
        if g == 0:
            nc.vector.tensor_copy(out=oacc[qs8][:], in_=o_ps[:])
        else:
            nc.vector.tensor_add(out=oacc[qs8][:], in0=oacc[qs8][:], in1=o_ps[:])


def _build():
    nc = bacc.Bacc()
    xb_d = nc.declare_dram_parameter("xb", [128, N], F32, isOutput=False)
    wpack_d = nc.declare_dram_parameter("wpack", [128, 5, 128], BF16, isOutput=False)
    cpack_d = nc.declare_dram_parameter("cpack", [128, 6], F32, isOutput=False)
    out_d = nc.declare_dram_parameter("out", [128, NQ], F32, isOutput=True)

    with tile.TileContext(nc) as tc:
        from contextlib import ExitStack

        with ExitStack() as ctx:
            big = ctx.enter_context(tc.tile_pool(name="big", bufs=1))
            mini = ctx.enter_context(tc.tile_pool(name="mini", bufs=2))
            ppool = ctx.enter_context(tc.tile_pool(name="pp", bufs=2))
            spool = ctx.enter_context(tc.tile_pool(name="sp", bufs=2, space="PSUM"))
            opool = ctx.enter_context(tc.tile_pool(name="op", bufs=2, space="PSUM"))
            mpsum = ctx.enter_context(tc.tile_pool(name="mp", bufs=2, space="PSUM"))

            xb_sb = big.tile([128, N], F32, tag="xb")
            h_sb = big.tile([128, N], BF16, tag="h")
            k_sb = big.tile([128, N], BF16, tag="k")
            q_sb = big.tile([128, NQ], BF16, tag="q")
            vt_sb = big.tile([128, 32, 129], BF16, tag="vt")
            wpack_sb = big.tile([128, 5, 128], BF16, tag="wpk")
            cpack_sb = big.tile([128, 6], F32, tag="cpk")
            ident = big.tile([128, 128], BF16, tag="id")
            ot_sb = big.tile([128, NQ], BF16, tag="ot")
            out_sb = big.tile([128, NQ], F32, tag="os")
            oacc = [
                big.tile([128, 129], F32, tag=f"oa{i}", name=f"oa{i}")
                for i in range(8)
            ]
            stats_sb = big.tile([128, 8, 6], F32, tag="bns")
            mv_sb = big.tile([128, 2], F32, tag="mv")
            stats_bf = big.tile([128, 2], BF16, tag="sbf")
            scale_col = big.tile([128, 1], F32, tag="scl")
            bias_col = big.tile([128, 1], F32, tag="bcl")

            # --- loads ---
            nc.sync.dma_start(out=wpack_sb[:], in_=wpack_d[:])
            nc.sync.dma_start(out=cpack_sb[:], in_=cpack_d[:])
            make_identity(nc, ident[:])
            for i in range(2):
                nc.sync.dma_start(
                    out=xb_sb[:, i * 2048 : (i + 1) * 2048],
                    in_=xb_d[:, i * 2048 : (i + 1) * 2048],
                )

            # --- GroupNorm stats ---
            # per-channel mean/var over N via bn_stats/bn_aggr
            for i in range(8):
                nc.vector.bn_stats(
                    out=stats_sb[:, i, :], in_=xb_sb[:, i * 512 : (i + 1) * 512]
                )
            nc.vector.bn_aggr(out=mv_sb[:], in_=stats_sb[:])
            # stats_bf = [mean_c, E_c[x^2]] in bf16 (E[x^2] = var + mean^2)
            msq = mini.tile([128, 1], F32, tag="msq")
            nc.vector.tensor_mul(out=msq[:], in0=mv_sb[:, 0:1], in1=mv_sb[:, 0:1])
            nc.vector.tensor_copy(out=stats_bf[:, 0:1], in_=mv_sb[:, 0:1])
            nc.vector.tensor_add(out=stats_bf[:, 1:2], in0=mv_sb[:, 1:2], in1=msq[:])
            # group-combine + broadcast back to channels: gmat is block-diag 1/(4N)
            st_ps = mpsum.tile([128, 2], F32, tag="mm")
            nc.tensor.matmul(
                st_ps[:], lhsT=wpack_sb[:, 4, :], rhs=stats_bf[:], start=True, stop=True
            )
            # var_g = E_g[x^2] - mean_g^2 ; rstd = 1/sqrt(var_g + eps)
            stg_sb = mini.tile([128, 2], F32, tag="stg")
            nc.vector.tensor_copy(out=stg_sb[:], in_=st_ps[:])
            msq2 = mini.tile([128, 1], F32, tag="msq2")
            varg = mini.tile([128, 1], F32, tag="varg")
            nc.vector.tensor_mul(out=msq2[:], in0=stg_sb[:, 0:1], in1=stg_sb[:, 0:1])
            nc.vector.tensor_sub(out=varg[:], in0=stg_sb[:, 1:2], in1=msq2[:])
            eps_col = big.tile([128, 1], F32, tag="eps")
            nc.vector.memset(eps_col[:], EPS)
            zero_col = big.tile([128, 1], F32, tag="zc")
            nc.vector.memset(zero_col[:], 0.0)
            stdg = mini.tile([128, 1], F32, tag="stdg")
            nc.scalar.activation(out=stdg[:], in_=varg[:], func=AF.Sqrt, bias=eps_col[:])
            rstd = mini.tile([128, 1], F32, tag="rstd")
            nc.vector.reciprocal(out=rstd[:], in_=stdg[:])
            # scale = rstd * gamma ; bias = beta - mean_g * scale
            nc.vector.tensor_mul(out=scale_col[:], in0=rstd[:], in1=cpack_sb[:, 0:1])
            tmpc = mini.tile([128, 1], F32, tag="tmpc")
            nc.vector.tensor_mul(out=tmpc[:], in0=stg_sb[:, 0:1], in1=scale_col[:])
            nc.vector.tensor_sub(out=bias_col[:], in0=cpack_sb[:, 1:2], in1=tmpc[:])
            # h = x * scale + bias  (bf16)
            for i in range(4):
                nc.vector.tensor_scalar(
                    out=h_sb[:, i * 1024 : (i + 1) * 1024],
                    in0=xb_sb[:, i * 1024 : (i + 1) * 1024],
                    scalar1=scale_col[:],
                    scalar2=bias_col[:],
                    op0=OP.mult,
                    op1=OP.add,
                )

            # --- K, Q', V^T ---
            for i in range(8):
                kq = mpsum.tile([128, 512], F32, tag="mm")
                nc.tensor.matmul(
                    kq[:],
                    lhsT=wpack_sb[:, 0, :],
                    rhs=h_sb[:, i * 512 : (i + 1) * 512],
                    start=True,
                    stop=True,
                )
                nc.vector.tensor_scalar_add(
                    out=k_sb[:, i * 512 : (i + 1) * 512],
                    in0=kq[:],
                    scalar1=cpack_sb[:, 3:4],
                )
            for i in range(2):
                kq = mpsum.tile([128, 512], F32, tag="mm")
                nc.tensor.matmul(
                    kq[:],
                    lhsT=wpack_sb[:, 1, :],
                    rhs=h_sb[:, i * 512 : (i + 1) * 512],
                    start=True,
                    stop=True,
                )
                nc.vector.tensor_scalar_add(
                    out=q_sb[:, i * 512 : (i + 1) * 512],
                    in0=kq[:],
                    scalar1=cpack_sb[:, 2:3],
                )
            for t in range(32):
                vt = mpsum.tile([128, 128], F32, tag="mm")
                nc.tensor.matmul(
                    vt[:],
                    lhsT=h_sb[:, t * 128 : (t + 1) * 128],
                    rhs=wpack_sb[:, 2, :],
                    start=True,
                    stop=True,
                )
                nc.vector.tensor_copy(out=vt_sb[:, t, 0:128], in_=vt[:])
            nc.vector.memset(vt_sb[:, :, 128:129], 1.0)

            # --- attention: S^T tiles -> exp -> O accumulation ---
            # software-pipelined by one key-group so exp(g) overlaps O(g-1)
            pprev = None
            for g in range(4):
                pcur = []
                for j in range(8):
                    kt = g * 8 + j
                    s_ps = spool.tile([128, 1024], F32, tag="s")
                    for half in range(2):
                        nc.tensor.matmul(
                            s_ps[:, half * 512 : (half + 1) * 512],
                            lhsT=k_sb[:, kt * 128 : (kt + 1) * 128],
                            rhs=q_sb[:, half * 512 : (half + 1) * 512],
                            start=True,
                            stop=True,
                        )
                    p = ppool.tile([128, 1024], BF16, tag=f"p{j}")
                    nc.scalar.activation(
                        out=p[:], in_=s_ps[:], func=AF.Exp, bias=zero_col[:]
                    )
                    pcur.append(p)
                if pprev is not None:
                    _emit_o_group(nc, opool, oacc, pprev, vt_sb, g - 1)
                pprev = pcur
            _emit_o_group(nc, opool, oacc, pprev, vt_sb, 3)

            # --- normalize, transpose, project, residual ---
            for qs8 in range(8):
                rden = mini.tile([128, 1], F32, tag="rden")
                nc.vector.reciprocal(out=rden[:], in_=oacc[qs8][:, 128:129])
                on_sb = mini.tile([128, 128], BF16, tag="on")
                nc.vector.tensor_scalar_mul(
                    out=on_sb[:], in0=oacc[qs8][:, 0:128], scalar1=rden[:]
                )
                tp_ps = mpsum.tile([128, 128], BF16, tag="mm")
                nc.tensor.transpose(out=tp_ps[:], in_=on_sb[:], identity=ident[:])
                nc.vector.tensor_scalar_add(
                    out=ot_sb[:, qs8 * 128 : (qs8 + 1) * 128],
                    in0=tp_ps[:],
                    scalar1=cpack_sb[:, 4:5],
                )
            for i in range(2):
                fin = mpsum.tile([128, 512], F32, tag="mm")
                nc.tensor.matmul(
                    fin[:],
                    lhsT=wpack_sb[:, 3, :],
                    rhs=ot_sb[:, i * 512 : (i + 1) * 512],
                    start=True,
                    stop=True,
                )
                nc.vector.tensor_scalar_add(
                    out=fin[:], in0=fin[:], scalar1=cpack_sb[:, 5:6]
                )
                nc.vector.tensor_add(
                    out=out_sb[:, i * 512 : (i + 1) * 512],
                    in0=fin[:],
                    in1=xb_sb[:, i * 512 : (i + 1) * 512],
                )
            nc.sync.dma_start(out=out_d[:], in_=out_sb[:])

    nc.finalize()
    return nc


_CACHED = None


def _get_nc():
    global _CACHED
    if _CACHED is None:
        _CACHED = _build()
    return _CACHED


def _prep_inputs(x, gn_w, gn_b, wq, bq, wk, bk, wv, bv, wp, bp):
    npbf = mybir.dt.np(BF16)
    s = float(C) ** -0.5
    wkT = np.ascontiguousarray(np.asarray(wk, np.float32).T).astype(npbf)
    wqTs = np.ascontiguousarray(np.asarray(wq, np.float32).T * s).astype(npbf)
    wvT = np.ascontiguousarray(np.asarray(wv, np.float32).T).astype(npbf)
    wpT = np.ascontiguousarray(np.asarray(wp, np.float32).T).astype(npbf)
    # bn_stats gives per-channel means; group stats = average over the
    # gs channels of the group (block-diagonal averaging matrix).
    gmat = np.zeros((C, C), np.float32)
    gs = C // GROUPS  # channels per group
    for g in range(GROUPS):
        gmat[g * gs : (g + 1) * gs, g * gs : (g + 1) * gs] = 1.0 / gs
    gmatb = gmat.astype(npbf)
    wpack = np.ascontiguousarray(
        np.stack([wkT, wqTs, wvT, wpT, gmatb], axis=1)
    )  # [128, 5, 128]
    cpack = np.ascontiguousarray(
        np.stack(
            [
                np.asarray(gn_w, np.float32),
                np.asarray(gn_b, np.float32),
                np.asarray(bq, np.float32) * s,
                np.asarray(bk, np.float32),
                np.asarray(bv, np.float32),
                np.asarray(bp, np.float32),
            ],
            axis=1,
        )
    )  # [128, 6]
    xf = np.asarray(x, np.float32).reshape(B, C, N)
    in_maps = []
    for c in range(NCORES):
        b, q4 = divmod(c, 4)
        qs = q4 * NQ
        xb = np.roll(xf[b], -qs, axis=1) if qs else xf[b]
        in_maps.append(
            {
                "xb": np.ascontiguousarray(xb),
                "wpack": wpack,
                "cpack": cpack,
            }
        )
    return in_maps


def _run(inputs, trace=False):
    nc = _get_nc()
    in_maps = _prep_inputs(**inputs)
    res = run_bass_kernel_spmd(
        nc, in_maps, core_ids=list(range(NCORES)), trace=trace
    )
    out = np.empty((B, C, N), np.float32)
    for c in range(NCORES):
        b, q4 = divmod(c, 4)
        out[b][:, q4 * NQ : (q4 + 1) * NQ] = res.results[c]["out"]
    return out.reshape(B, C, 16, 16, 16), res


def kernel(**inputs):
    out, _ = _run(inputs, trace=False)
    return out
